# revision 60
# baseline (speedup 1.0000x reference)
"""Trainium2 Bass kernel for the ODLUE path-flow model (nn_AESUELOGIT).

Math (per reference):
  V[b,l]   = sum_f X[b,l,1+f]*theta[f] + theta_links[l]        (b = day*hour, 96)
  Vf[b,p]  = sum_l V[b,l]*D[l,p] + psc*log(psf[p])
  pf       = per-OD softmax over each OD's 4 consecutive paths
  f[b,p]   = pf * sqrt_q[od(p)]**2
  out[b,l] = relu(sum_p f[b,p]*D[l,p])

Distribution: shard the path axis P=20000 across 8 cores (2500 paths =
625 ODs per core; OD groups of 4 stay device-local). Each core computes
a partial link flow over its paths; host sums partials + relu.

Per-core dataflow (v3 — DMA-delivery-bound fix):
  v2 put the whole 10.3 MB D/D^T stream on ONE HWDGE ring in 36 x 655KB
  transfers (5KB descriptors): measured ~213 GB/s with the ring idle
  44% of its span, so matmul1 could not start until ~24us and the whole
  kernel tracked the single serialized stream (61us).
  v3 splits the stream across BOTH HWDGE rings (sync + scalar), which
  the 16 SDMA engines drain round-robin at packet granularity, and
  enlarges transfers to ~1MB with 8KB/partition descriptors (measured
  ~341+ GB/s regime):
    sync   : X[tiles 0-7], D-chunk 0,2,4, DT-chunk 1,3, out 0,2
    scalar : thtl, X[tiles 8-15], D-chunk 1,3, DT-chunk 0,2,4, out 1,3
  D is repacked chunk-major in DRAM ([NPC,128,NLT,PC] — one 512-path
  chunk = full K contiguous per partition) so matmul1 runs chunk-outer:
  chunk n's 16 DR-matmuls start as soon as its 1.05MB lands, ~3.0us
  per chunk behind the stream, softmax per chunk right behind. Pad
  rows/cols are zeros in DRAM (no memsets, no partial transfers).
  The two big matmuls keep the v2 shape (measured optimal: fp8
  DoubleRow, FD=512/500 streams, V split hi+lo fp8 accumulating in one
  fp32 PSUM bank ~ bf16 accuracy at 213ns/512-col matmul).
  mm2 + transposes + staggered output identical to v2.

Host prep (layout/sharding only): X channel 0 stripped and packed to
partition-major l-tiles [128, NLT, CH, B], D cast to fp8 chunk-major
[NPC, 128, NLT, PC] (plus transposed copy [128, NPT, L]), per-core
slices of D/sqrt_q; host sums the 8 bf16 partials in f32 + relu.
"""

import sys
import types

import ml_dtypes
import numpy as np

# --- NTFF profile hook shim (missing antenv.axon_hooks in this image) ---
try:
    import antenv

    if "antenv.axon_hooks" not in sys.modules:
        _m = types.ModuleType("antenv.axon_hooks")
        _state = {}
        _m.set_axon_ntff_profile_hook = lambda h: _state.__setitem__("h", h)
        _m.get_axon_ntff_profile_hook = lambda: _state.get("h")
        sys.modules["antenv.axon_hooks"] = _m
        antenv.axon_hooks = _m
        try:
            from trn_agent_boot.trn_boot import _ntff_profile_via_ctypes

            _m.set_axon_ntff_profile_hook(
                _ntff_profile_via_ctypes("/opt/axon/libaxon_pjrt.so")
            )
        except Exception:
            pass
except Exception:
    pass

import concourse.bass as bass
import concourse.mybir as mybir
import concourse.tile as tile
from concourse import bacc
from concourse.bass import ds, ts
from concourse.bass_utils import run_bass_kernel_spmd
from concourse.masks import make_identity

BF = mybir.dt.bfloat16
F32 = mybir.dt.float32
FP8 = mybir.dt.float8e4
AF = mybir.ActivationFunctionType
ALU = mybir.AluOpType
AX = mybir.AxisListType
DR = mybir.MatmulPerfMode.DoubleRow

NCORES = 8
B = 96           # n_days * n_hours
L = 2000         # links
CH = 4           # X feature channels (ch 0 = tt_ff stripped on host)
P = 20000        # paths
PPG = 4          # paths per OD
PL = P // NCORES          # 2500 local paths
GL = PL // PPG            # 625 local ODs
NLT = 16                  # l-tiles (l padded to 2048 with zero D rows)
NPT = 20                  # p-tiles (p padded to 2560 with zero D^T rows)
PLP = NPT * 128           # 2560 padded local paths (pad: D cols 0, q 0)
GLP = PLP // PPG          # 640 padded local ODs
LP = NLT * 128            # 2048 padded links (K of matmul1 only)
NPC = 5                   # matmul1 psum chunks
PC = PLP // NPC           # 512 (exactly one PSUM bank)
PCG = PC // PPG           # 128 groups per chunk
NLC = 4                   # matmul2 psum chunks
LC = L // NLC             # 500 exact links per chunk (no pad in matmul2 N)
NDT = 5                   # dtsb transfer chunks (4 p-tiles each)

_CACHE = {}


def _build_nc(theta, with_bias=True):
    # theta compiled as immediates: the runtime-scalar DVE op
    # (InstTensorScalarPtr) runs at 1 elem/cycle; immediates may unlock
    # the packed 2x mode, and the kernel is compiled per theta anyway
    theta = tuple(float(t) for t in theta)
    key = ("nc", theta, with_bias)
    if key in _CACHE:
        return _CACHE[key]
    nc = bacc.Bacc()

    # X packed group-major and channel-major ([128, 2 groups, CH, 8
    # l-tiles, B]) so every V-chain operand is one contiguous per-
    # partition run
    xp_ext = nc.declare_dram_parameter(
        "xp", [128, 2, CH, NLT // 2, B], BF, isOutput=False
    )
    # theta_links [:, :NLT] and theta [:, NLT:] packed into one per-partition
    # tensor: a single 128-element HWDGE transfer instead of slow SWDGE
    # broadcasts gating the V-chain
    thtl_ext = nc.declare_dram_parameter("thtl", [128, NLT + CH], BF, isOutput=False)
    # D chunk-major, split in two K-halves (l-tiles 0-7 / 8-15): chunk
    # (c, h) holds half the K for paths [512c, 512c+512)
    d_ext = nc.declare_dram_parameter(
        "dloc", [NPC, 2, 128, NLT // 2, PC], FP8, isOutput=False
    )
    dt_ext = nc.declare_dram_parameter("dtloc", [128, NPT, L], FP8, isOutput=False)
    sq_ext = nc.declare_dram_parameter("sq", [1, GLP], F32, isOutput=False)
    if with_bias:
        psf_ext = nc.declare_dram_parameter("psf", [1, PLP], F32, isOutput=False)
        psc_ext = nc.declare_dram_parameter("psc", [1, 1], F32, isOutput=False)
    out_ext = nc.declare_dram_parameter("out", [B, L], BF, isOutput=True)

    with tile.TileContext(nc) as tc:
        with (
            tc.tile_pool(name="const", bufs=1) as const,
            tc.tile_pool(name="work", bufs=1) as work,
        ):
            # ---- big loads: ~1MB transfers (8KB/partition descriptors),
            # split across BOTH HWDGE rings in consumption order; the 16
            # SDMA engines round-robin the two rings at packet
            # granularity so the two streams drain concurrently at the
            # HBM cap. Pad rows/cols are zeros in DRAM.  ----
            xq_sb = work.tile([128, 2, CH, NLT // 2, B], BF)
            dsb = work.tile([128, NPC, NLT, PC], FP8)
            dtsb = work.tile([128, NPT, L], FP8)
            thtl_sb = const.tile([128, NLT + CH], BF)

            XH = NLT // 2
            # The big load rides BOTH HWDGE rings as one strict
            # consumption-ordered stream.  Measured ring behavior: each
            # ring drains its transfers SERIALLY (next starts ~0.3us after
            # the previous' data), the two rings overlap at ~420 GB/s
            # aggregate, and any third queue with pending work (e.g. a
            # SWDGE broadcast) can serialize the whole pool at transfer
            # granularity — so X leads BOTH rings, everything else lands
            # behind it in consumption order, and the only SWDGE loads are
            # two tiny rows:
            #   sync   : thtl (tiny), X1, d0a..d4a, dt0, dt2, out0, out2
            #   scalar : qrow (tiny), X2, d0b..d4b, dt1, dt3, dt4, out1/3
            # Measured: the first transfer of each ring serializes against
            # the other ring's first; making BOTH firsts tiny unlocks
            # two-ring concurrency from ~9us so the X halves land together.
            # D halves split by K (l-tiles 0-7 on sync / 8-15 on scalar),
            # aligned with the X halves and V-chain groups so matmul1's
            # k-pairs 0-3 of a chunk depend only on the sync half.
            # Sync-queue DMA-gens may stall on completion-sem lane
            # recycling (queue is DMA-only, harmless, self-pacing);
            # scalar-ring gens past the 4th are placed further down at
            # program points where their lane wait is provably already
            # satisfied, so they never block a time-critical ACT op
            # (v3 lost 8us and v4 12us to exactly that).
            qrow_sb = const.tile([1, GLP], F32)
            nc.sync.dma_start(out=thtl_sb, in_=thtl_ext[:])
            nc.sync.dma_start(out=xq_sb[:, 0], in_=xp_ext[:, 0])
            nc.scalar.dma_start(out=qrow_sb, in_=sq_ext[:])
            nc.scalar.dma_start(out=xq_sb[:, 1], in_=xp_ext[:, 1])
            nc.sync.dma_start(out=dsb[:, 0, :XH], in_=d_ext[0, 0])
            nc.scalar.dma_start(out=dsb[:, 0, XH:], in_=d_ext[0, 1])
            nc.sync.dma_start(out=dsb[:, 1, :XH], in_=d_ext[1, 0])
            nc.scalar.dma_start(out=dsb[:, 1, XH:], in_=d_ext[1, 1])
            nc.sync.dma_start(out=dsb[:, 2, :XH], in_=d_ext[2, 0])
            nc.sync.dma_start(out=dsb[:, 3, :XH], in_=d_ext[3, 0])
            nc.sync.dma_start(out=dsb[:, 4, :XH], in_=d_ext[4, 0])
            nc.sync.dma_start(out=dtsb[:, 0:4], in_=dt_ext[:, 0:4])
            nc.sync.dma_start(out=dtsb[:, 8:12], in_=dt_ext[:, 8:12])

            tl_sb = thtl_sb[:, :NLT]
            if with_bias:
                psf_sb = const.tile([1, PLP], F32)
                nc.gpsimd.dma_start(out=psf_sb, in_=psf_ext[:])
                psc_sb = const.tile([1, 1], F32)
                nc.gpsimd.dma_start(out=psc_sb, in_=psc_ext[:])
            ones_sb = const.tile([1, B], BF)
            nc.vector.memset(ones_sb, 1.0)
            # warm the ACT Exp table early so softmax exp chunks don't pay
            # the ~1.4us table load mid-kernel
            dummy = const.tile([1, 8], F32)
            nc.vector.memset(dummy, 0.0)
            nc.scalar.activation(out=dummy, in_=dummy, func=AF.Exp)
            # scalar-ring d2b gen here: its sem-lane recycle wait clears
            # well before the vthi casts behind it on the ACT queue
            nc.scalar.dma_start(out=dsb[:, 2, XH:], in_=d_ext[2, 1])
            ident = const.tile([128, 128], BF)
            make_identity(nc, ident)
            # pre-warm the PE clock gate (HAM, 4096-cycle window): a dense
            # block long enough to flip HAM to 8/8 before matmul1's data
            # lands, then X- and vthi-chained matmuls to keep every PE gap
            # under the ~3.4us re-throttle window up to the first real MM
            warm = const.tile([128, 512], BF)
            nc.vector.memset(warm, 0.0)
            pwcm = tc.tile_pool(name="pswarm", bufs=1, space="PSUM")
            pwp = pwcm.__enter__()
            pw = pwp.tile([128, 512], F32)
            for _ in range(10):
                nc.tensor.matmul(
                    pw[:96], lhsT=warm[:, :96], rhs=warm[:, :512],
                    start=True, stop=True,
                )

            # qb = sqrt_q**2 broadcast over batch partitions WITHOUT a
            # SWDGE broadcast DMA (measured: pending SWDGE bulk work
            # serializes the whole SDMA pool at the worst moment): square
            # the row on one ACT lane (after the vthi casts — it waits on
            # the slow-landing SWDGE row), then rank-1 matmul ones x qsq
            # through the warm PSUM bank, slotted between mm1 chunks so
            # nothing time-critical ever waits on this chain.
            qb = const.tile([128, GLP], F32)
            qsqb = const.tile([1, GLP], BF)

            # ---- V^T tiles (l on partitions), bf16 chain on DVE ----
            # vtf = X_c0*th0 + tl; vtf += X_c*th_c (c=1,2,3);
            # vthi = fp8(vtf) on ACT; vtlo = fp8(vtf - vthi) on DVE.
            vtf = work.tile([128, NLT, B], BF)
            vthi = work.tile([128, NLT, B], FP8)
            vtlo = work.tile([128, NLT, B], FP8)
            for g in range(2):
                tsl = slice(g * XH, (g + 1) * XH)
                tl_sl = tl_sb[:, tsl]
                tl_rep = bass.AP(
                    tensor=tl_sl.tensor,
                    offset=tl_sl.offset,
                    ap=[tl_sl.ap[0], tl_sl.ap[1], [0, B]],
                )
                nc.vector.scalar_tensor_tensor(
                    out=vtf[:, tsl], in0=xq_sb[:, g, 0],
                    scalar=theta[0],
                    in1=tl_rep, op0=ALU.mult, op1=ALU.add,
                )
                for c in (1, 2, 3):
                    nc.vector.scalar_tensor_tensor(
                        out=vtf[:, tsl], in0=xq_sb[:, g, c],
                        scalar=theta[c],
                        in1=vtf[:, tsl], op0=ALU.mult, op1=ALU.add,
                    )
                nc.scalar.copy(out=vthi[:, tsl], in_=vtf[:, tsl])
                nc.vector.tensor_sub(vtlo[:, tsl], vtf[:, tsl], vthi[:, tsl])
                if g == 0:
                    # keep the PE HAM window alive up to matmul1's gate
                    # (d0 arrival): an X1-chained pair, a further dense
                    # stretch, then pairs chained to the g0 cast and X2 —
                    # every PE gap stays under the ~3.4us re-throttle
                    # window and NOTHING is chained past the d0 gate
                    # (measured: warm matmuls chained to D-chunks sat in
                    # the PE FIFO at the critical moment, costing ~2.5us)
                    for _ in range(2):
                        nc.tensor.matmul(
                            pw[:96], lhsT=xq_sb[:, 0, 0, 0, :],
                            rhs=warm[:, :512],
                            start=True, stop=True,
                        )
                    for _ in range(6):
                        nc.tensor.matmul(
                            pw[:96], lhsT=warm[:, :96], rhs=warm[:, :512],
                            start=True, stop=True,
                        )
                    for _ in range(2):
                        nc.tensor.matmul(
                            pw[:96], lhsT=xq_sb[:, 1, 0, 0, :],
                            rhs=warm[:, :512],
                            start=True, stop=True,
                        )
                    for _ in range(2):
                        nc.tensor.matmul(
                            pw[:96], lhsT=warm[:, :96], rhs=warm[:, :512],
                            start=True, stop=True,
                        )
                    for _ in range(2):
                        nc.tensor.matmul(
                            pw[:96], lhsT=vthi[:, 0, :], rhs=warm[:, :512],
                            start=True, stop=True,
                        )

            # scalar-ring D second halves: gens recycle early-stream sem
            # lanes; issued after the vthi casts so they cannot delay them
            nc.scalar.dma_start(out=dsb[:, 3, XH:], in_=d_ext[3, 1])
            nc.scalar.dma_start(out=dsb[:, 4, XH:], in_=d_ext[4, 1])
            # q row squared (BF) — waits on the SWDGE row; after the casts
            nc.scalar.activation(out=qsqb, in_=qrow_sb, func=AF.Square)

            if with_bias:
                # crow = psc * ln(psf)  (bf16 row, folded into matmul1 as K=1)
                lnp = const.tile([1, PLP], F32)
                nc.scalar.activation(out=lnp, in_=psf_sb, func=AF.Ln)
                crow = const.tile([1, PLP], BF)
                nc.vector.tensor_scalar_mul(crow, lnp, psc_sb[:, 0:1])

            e_sb = work.tile([128, PLP], BF)
            f_sb = work.tile([128, PLP], BF)
            s_sb = work.tile([128, GLP], F32)
            r_sb = work.tile([128, GLP], F32)
            t_sb = work.tile([128, GLP], BF)
            fT8 = work.tile([128, NPT, B], FP8)

            def _softmax_chunk(n, ps1):
                nc.scalar.activation(
                    out=e_sb[:B, ts(n, PC)], in_=ps1[n][:B], func=AF.Exp
                )
                e3 = e_sb[:B, ts(n, PC)].rearrange("p (g w) -> p g w", w=PPG)
                nc.vector.reduce_sum(
                    out=s_sb[:B, ds(n * PCG, PCG)], in_=e3, axis=AX.X
                )
                nc.vector.reciprocal_approx_fast(
                    out=r_sb[:B, ds(n * PCG, PCG)],
                    in_=s_sb[:B, ds(n * PCG, PCG)],
                )
                nc.vector.tensor_mul(
                    t_sb[:B, ds(n * PCG, PCG)],
                    r_sb[:B, ds(n * PCG, PCG)],
                    qb[:B, ds(n * PCG, PCG)],
                )
                t_sl = t_sb[:B, ds(n * PCG, PCG)]
                t_rep = bass.AP(
                    tensor=t_sl.tensor,
                    offset=t_sl.offset,
                    ap=[t_sl.ap[0], t_sl.ap[1], [0, PPG]],
                )
                f3 = f_sb[:B, ts(n, PC)].rearrange("p (g w) -> p g w", w=PPG)
                nc.vector.tensor_tensor(out=f3, in0=e3, in1=t_rep, op=ALU.mult)

            with tc.tile_pool(name="psT", bufs=2, space="PSUM") as psTp:

                def _transp_chunk(c):
                    # transpose chunk c's 4 p-tiles into one PSUM bank, then
                    # a single ACT copy to fp8
                    pT = psTp.tile([128, 4, B], BF)
                    for k in range(4):
                        j = 4 * c + k
                        nc.tensor.transpose(
                            pT[:, k, :], f_sb[:B, ds(128 * j, 128)], ident[:B, :B]
                        )
                    nc.scalar.copy(out=fT8[:, 4 * c : 4 * c + 4, :], in_=pT)

                # ---- matmul1, chunk-outer: chunk n accumulates all 8
                # DR k-pairs (hi+lo) into one PSUM bank right behind its
                # 1.05MB DMA chunk; softmax overlaps the next chunk ----
                ps1cm = tc.tile_pool(name="ps1", bufs=1, space="PSUM")
                ps1p = ps1cm.__enter__()
                ps1 = [
                    ps1p.tile([128, PC], F32, name=f"ps1_{n}", tag=f"b{n}")
                    for n in range(NPC)
                ]
                NG = NLT // 2
                for n in range(NPC):
                    # four quarter-blocks (hi/lo x K-halves): the late
                    # vtlo / g1 dependencies sit at block boundaries so
                    # early k-pairs never stall behind them in PE order
                    blocks = [
                        (vthi, range(0, NG // 2)),
                        (vtlo, range(0, NG // 2)),
                        (vthi, range(NG // 2, NG)),
                        (vtlo, range(NG // 2, NG)),
                    ]
                    for bi, (vsrc, grange) in enumerate(blocks):
                        for g in grange:
                            gsl = slice(2 * g, 2 * g + 2)
                            nc.tensor.matmul(
                                ps1[n][:B],
                                lhsT=vsrc[:, gsl, :],
                                rhs=dsb[:, n, gsl, :],
                                start=(bi == 0 and g == 0),
                                stop=(
                                    not with_bias and bi == 3 and g == NG - 1
                                ),
                                perf_mode=DR,
                            )
                    if with_bias:
                        nc.tensor.matmul(
                            ps1[n][:B], lhsT=ones_sb[:1, :],
                            rhs=crow[:1, ts(n, PC)],
                            start=False, stop=True, skip_group_check=True,
                        )
                    # qb rank-1 broadcasts ride the PE between chunks; the
                    # DVE copy lands just ahead of the softmax that reads it
                    if n == 0:
                        nc.tensor.matmul(
                            pw[:B, :512], lhsT=ones_sb[:1, :],
                            rhs=qsqb[:1, :512],
                            start=True, stop=True, skip_group_check=True,
                        )
                        nc.vector.tensor_copy(
                            out=qb[:B, :512], in_=pw[:B, :512]
                        )
                    elif n == 1:
                        nc.tensor.matmul(
                            pw[:B, :128], lhsT=ones_sb[:1, :],
                            rhs=qsqb[:1, 512:],
                            start=True, stop=True, skip_group_check=True,
                        )
                        nc.vector.tensor_copy(
                            out=qb[:B, 512:], in_=pw[:B, :128]
                        )
                    _softmax_chunk(n, ps1)
                    # scalar-ring DT chunks 1/3/4, slotted between softmax
                    # chunks so their sem-lane-recycle waits never block a
                    # time-critical ACT op
                    if n == 0:
                        nc.scalar.dma_start(
                            out=dtsb[:, 4:8], in_=dt_ext[:, 4:8]
                        )
                    elif n == 1:
                        nc.scalar.dma_start(
                            out=dtsb[:, 12:16], in_=dt_ext[:, 12:16]
                        )
                    elif n == 2:
                        nc.scalar.dma_start(
                            out=dtsb[:, 16:20], in_=dt_ext[:, 16:20]
                        )
                    # transposes ride two chunks behind matmul1 (their
                    # f_sb is long ready, so the PE never waits on the
                    # softmax chain) — fT8 is fully built right after mm1
                    # instead of serializing the whole exp->copy->transpose
                    # chain at the mm1->mm2 junction (~6us measured)
                    if n >= 2:
                        _transp_chunk(n - 2)
                _transp_chunk(3)
                _transp_chunk(4)
                ps1cm.__exit__(None, None, None)

                # ---- matmul2 interleaved with the remaining transposes so
                # the PE stream never idles: pairs 0..5 need only chunks
                # 0..2 of f^T; chunk 3/4 transposes slot in between ----
                out_sb = work.tile([128, L], BF)
                with tc.tile_pool(name="ps2", bufs=1, space="PSUM") as ps2p:
                    ps2 = [
                        ps2p.tile([128, LC], F32, name=f"ps2_{m}", tag=f"c{m}")
                        for m in range(NLC)
                    ]
                    NJ = NPT // 2

                    # m-major: each 500-link chunk runs its full K
                    # accumulation back-to-back, so its copy + out-DMA
                    # overlap the remaining chunks' matmuls and only the
                    # last chunk's drain trails the final matmul
                    for m in range(NLC):
                        for j in range(NJ):
                            gsl = slice(2 * j, 2 * j + 2)
                            nc.tensor.matmul(
                                ps2[m][:B],
                                lhsT=fT8[:, gsl, :],
                                rhs=dtsb[:, gsl, ts(m, LC)],
                                start=(j == 0), stop=(j == NJ - 1),
                                perf_mode=DR,
                            )
                        if m % 2 == 0:
                            nc.vector.tensor_copy(
                                out=out_sb[:B, ts(m, LC)], in_=ps2[m][:B]
                            )
                            nc.sync.dma_start(
                                out=out_ext[:, ts(m, LC)],
                                in_=out_sb[:B, ts(m, LC)],
                            )
                        else:
                            nc.scalar.copy(
                                out=out_sb[:B, ts(m, LC)], in_=ps2[m][:B]
                            )
                            nc.scalar.dma_start(
                                out=out_ext[:, ts(m, LC)],
                                in_=out_sb[:B, ts(m, LC)],
                            )
            pwcm.__exit__(None, None, None)

    nc.finalize()
    _CACHE[key] = nc
    return nc


def _prep_inputs(X, theta, theta_links, sqrt_q, psf, psc_factor, D):
    bf = ml_dtypes.bfloat16
    fp8 = ml_dtypes.float8_e4m3
    f32 = np.float32

    # X packed group/channel-major: xp[p, g, c, t, b] = X[b, 128*(8g+t)+p, 1+c],
    # l zero-padded to 2048
    Xf = np.asarray(X, f32).reshape(B, L, CH + 1)[:, :, 1:]  # [B, L, CH]
    Xf = Xf.transpose(1, 2, 0)                               # [L, CH, B]
    Xpad = np.zeros((NLT * 128, CH, B), f32)
    Xpad[:L] = Xf
    xp = np.ascontiguousarray(
        Xpad.reshape(2, NLT // 2, 128, CH, B).transpose(2, 0, 3, 1, 4)
    ).astype(bf)  # [128, 2, CH, 8, B]

    tlp = np.zeros((NLT * 128,), f32)
    tlp[:L] = np.asarray(theta_links, f32)
    tlp = np.ascontiguousarray(tlp.reshape(NLT, 128).T)  # [128, NLT]
    thtl = np.zeros((128, NLT + CH), f32)
    thtl[:, :NLT] = tlp
    thtl = thtl.astype(bf)

    psc = np.asarray(psc_factor, f32).reshape(1, 1)
    D8 = np.asarray(D, f32).astype(fp8)  # exact for 0/1 entries

    in_maps = []
    for i in range(NCORES):
        pl = slice(i * PL, (i + 1) * PL)
        gl = slice(i * GL, (i + 1) * GL)
        sq_p = np.zeros((1, GLP), f32)
        sq_p[0, :GL] = np.asarray(sqrt_q, f32)[gl]
        dl = D8[:, pl]                                   # [2000, 2500]
        dpad = np.zeros((LP, PLP), fp8)
        dpad[:L, :PL] = dl
        # chunk-major, K-halved: dloc[c, h, p, t, j] = dpad[128*(8h+t)+p, 512c+j]
        dloc = np.ascontiguousarray(
            dpad.reshape(2, NLT // 2, 128, NPC, PC).transpose(3, 0, 2, 1, 4)
        )                                                # [NPC, 2, 128, 8, PC]
        dtpad = np.zeros((PLP, L), fp8)
        dtpad[:PL] = dl.T
        dtloc = np.ascontiguousarray(
            dtpad.reshape(NPT, 128, L).transpose(1, 0, 2)
        )                                                # [128, NPT, L]
        m = dict(xp=xp, thtl=thtl, dloc=dloc, dtloc=dtloc, sq=sq_p)
        if np.any(psc != 0.0):
            psf_p = np.ones((1, PLP), f32)
            psf_p[0, :PL] = np.asarray(psf, f32)[pl]
            m["psf"] = psf_p
            m["psc"] = psc
        in_maps.append(m)
    return in_maps


def run_on_cores(inputs, trace=False, **kw):
    """Compile (cached) + run SPMD on 8 cores; returns BassKernelResults."""
    with_bias = bool(np.any(np.asarray(inputs["psc_factor"], np.float32) != 0.0))
    nc = _build_nc(np.asarray(inputs["theta"], np.float32), with_bias=with_bias)
    in_maps = _prep_inputs(
        inputs["X"], inputs["theta"], inputs["theta_links"], inputs["sqrt_q"],
        inputs["psf"], inputs["psc_factor"], inputs["D"],
    )
    return run_bass_kernel_spmd(
        nc, in_maps, core_ids=list(range(NCORES)), trace=trace, **kw
    )


def kernel(X, theta, theta_links, sqrt_q, psf, psc_factor, D, path_od=None):
    res = run_on_cores(
        dict(X=X, theta=theta, theta_links=theta_links, sqrt_q=sqrt_q,
             psf=psf, psc_factor=psc_factor, D=D)
    )
    acc = np.zeros((B, L), np.float32)
    for r in res.results:
        acc += np.asarray(r["out"], np.float32)
    return np.maximum(acc, 0.0).reshape(4, 24, L)
